# revision 42
# baseline (speedup 1.0000x reference)
"""Trainium2 Bass kernel for nn_ModalityConsisLoss (8 NeuronCores, data-parallel).

Reference computation:
    v_spa/v_seq = concat([f[:,a,:], f[:,2,:]], -1) @ W + b   for a in (0,1,3)  -> [3B, D]
    z = normalize_rows(concat([v_spa, v_seq]))               -> [6B, D]
    sim = z @ z.T ;  pos = diag pairs (i, i+3B)
    loss = sum(-pos/T) + sum(log(rowsum(exp(sim/T)) - diag)) / (6B)

Strategy (data-parallel over B):
  Each core owns B/8 = 256 batch rows -> 1536 of the 12288 z-rows.
  Per core, per modality half (spa then seq):
    - load f shard (bf16), PE-transpose -> fT fp8, DoubleRow fp8 projection
      with W*16 in fp8 (normalization cancels every input scale)
    - column norms via ones-matmul; r = 16/sqrt(ssq) computed as
      exp(-0.5*ln(ssq) + ln 16) on ACT (all tables = natural_log_exp set,
      so the whole kernel needs one table load)
    - zT_half = fp8_e4m3(vT * r)  [512, 768]
    - AllGather the half (spa gather overlaps the seq prologue; a tiny
      warm-up AllGather at kernel start absorbs the CC-ring init + launch
      skew that otherwise put ~30us on the first real gather)
  sim tiles: DoubleRow fp8 matmuls, chunks of [128,2048] (4 PSUM banks,
  2 bufs) with fused exp(sim/(T*256)) + row-sum on ACT.
  Phase C (spa rows x seq cols) is reused for the seq rows' spa-column
  denominators: e tiles accumulate on DVE in bf16 ([128] lanes), one
  ones-matmul partition-reduce per 512-col block at the end, then a
  ReduceScatter whose rank-r shard is exactly our local seq rows.
  denom = rowsum - e^2 ; partial loss = sum(log denom) - (2/T)*sum(pos).
  Host sums the 8 partial scalars.
"""
import sys
from contextlib import ExitStack

sys.path.insert(0, "/opt/trn_rl_repo")

import numpy as np

import concourse.bass as bass
import concourse.mybir as mybir
import concourse.tile as tile
from concourse import bacc
from concourse import bass_utils
from concourse.masks import make_identity

F32 = mybir.dt.float32
BF16 = mybir.dt.bfloat16
FP8 = mybir.dt.float8e4
AF = mybir.ActivationFunctionType
ALU = mybir.AluOpType
DR = mybir.MatmulPerfMode.DoubleRow

N_CORES = 8
B = 2048
BL = B // N_CORES          # 256 local batch rows
D = 512
KB = D // 128              # 4 d blocks of 128
HROWS = 3 * BL             # 768 rows per modality half
LROWS = 2 * HROWS          # 1536 local z-rows (spa 768 | seq 768)
R = N_CORES * LROWS        # 12288 total rows
HALL = N_CORES * HROWS     # 6144 gathered columns per half
IB = LROWS // 128          # 12 row blocks of 128 per core
HIB = IB // 2
SIMW = 2048                # sim chunk width (4 PSUM banks, one ACT op)
CC = HALL // SIMW          # 3 sim column chunks per half
JT = SIMW // 512
MMW = 512                  # matmul moving width (single PSUM bank)
LH = (0, 1, 3)             # left heads of the pairs (x, 2)
TEMP = 0.5
ZSCALE = 16.0              # fp8 z scaling
WSCALE = 16.0              # fp8 W scaling (cancelled by normalization)
ESCALE = (1.0 / TEMP) / (ZSCALE * ZSCALE)
POS_COEF = (-2.0 / TEMP) / (ZSCALE * ZSCALE)
E2 = float(np.exp(2.0))    # diagonal term exp(2 * ||z||^2), ||z|| == 1
INV_COUNT = 1.0 / R        # final 1/(2*half)


def _body(ctx, nc, tc, f_aps, w_ap, b_ap, out_ap):
    const_pool = ctx.enter_context(tc.tile_pool(name="const", bufs=1))
    small_pool = ctx.enter_context(tc.tile_pool(name="small", bufs=1))
    vt_pool = ctx.enter_context(tc.tile_pool(name="vt", bufs=1))
    dram_pool = ctx.enter_context(tc.tile_pool(name="dram", bufs=1,
                                               space="DRAM"))
    big_pool = ctx.enter_context(tc.tile_pool(name="big", bufs=1))

    # ---- warm-up AllGather: pays the CC-ring init + rank launch skew
    # here (fully overlapped with the f/W loads and PE warm-up) so the
    # first real gather runs at steady-state speed.
    wz = const_pool.tile([1, 128], FP8)
    nc.gpsimd.memset(wz[:], 0.0)
    warm_ci = dram_pool.tile([128], FP8, tag="warm_ci")
    warm_co = dram_pool.tile([N_CORES * 128], FP8, addr_space="Shared",
                             tag="warm_co")
    nc.sync.dma_start(warm_ci[:], wz[:])
    nc.gpsimd.collective_compute(
        "AllGather", ALU.bypass,
        replica_groups=[list(range(N_CORES))],
        ins=[warm_ci.opt()], outs=[warm_co.opt()])

    ident_b = const_pool.tile([128, 128], BF16)
    make_identity(nc, ident_b[:])
    ones_col = const_pool.tile([128, 1], F32)
    nc.vector.memset(ones_col[:], 1.0)
    ones_col_b = const_pool.tile([128, 1], BF16)
    nc.vector.memset(ones_col_b[:], 1.0)
    ones_row_b = const_pool.tile([1, 128], BF16)
    nc.vector.memset(ones_row_b[:], 1.0)
    neg_e2 = const_pool.tile([128, 1], F32)
    nc.vector.memset(neg_e2[:], -E2)
    ln_zs = const_pool.tile([1, 1], F32)
    nc.vector.memset(ln_zs[:], float(np.log(ZSCALE)))

    # b columns: [128, 4] (per d_out block), scaled to match W*WSCALE
    b_col = const_pool.tile([128, 4], F32)
    for m in range(KB):
        nc.sync.dma_start(b_col[:, m:m + 1], b_ap[m * 128:(m + 1) * 128])
    nc.vector.tensor_scalar_mul(b_col[:], b_col[:], WSCALE)
    w_f8 = const_pool.tile([128, 8, D], FP8)

    vT = vt_pool.tile([128, KB, LROWS], F32)       # [d_out(blk,128), rows]
    zT_loc = small_pool.tile([128, KB, LROWS], FP8, tag="zT_loc")
    r_row = small_pool.tile([1, LROWS], BF16, tag="r_row")
    zT_all = [None, None]

    # sim-phase accumulators (memset early, off the critical path)
    stats = small_pool.tile([128, IB * 2 * CC], F32, tag="stats")
    nc.gpsimd.memset(stats[:], 0.0)
    acc_bf = small_pool.tile([128, CC, SIMW], BF16, tag="acc_bf")
    nc.gpsimd.memset(acc_bf[:], 0.0)

    with tc.tile_pool(name="fstage", bufs=4) as fst_pool, \
         tc.tile_pool(name="ftrans", bufs=1) as ft_pool, \
         tc.tile_pool(name="sq", bufs=2) as sq_pool, \
         tc.tile_pool(name="ps_t", bufs=2, space="PSUM") as ps_t, \
         tc.tile_pool(name="ps_proj", bufs=2, space="PSUM") as ps_proj, \
         tc.tile_pool(name="ps_s", bufs=2, space="PSUM") as ps_s:

        # PE warm-up while the f DMAs land: HAM holds the PE at low clock
        # until ~3.4us of sustained activity; chained dummy matmuls warm it.
        warm_sb = const_pool.tile([128, 512], BF16)
        nc.vector.memset(warm_sb[:], 0.0)
        wps = ps_proj.tile([128, 512], F32, name="wps", tag="psv")
        for _ in range(16):
            nc.tensor.matmul(wps[:], lhsT=warm_sb[:, 0:128],
                             rhs=warm_sb[:], start=True, stop=True)
        scrap = const_pool.tile([1, 1], F32)
        nc.vector.tensor_copy(scrap[:], wps[0:1, 0:1])
        nc.sync.dma_start(out_ap[:], scrap[:])

        # f loads first (the transposes need them); W lands during them.
        f_sts = {}
        for mod in range(2):
            for h in range(2):
                f_st = fst_pool.tile([128, 4 * D], F32,
                                     name=f"f_st{mod}{h}", tag="f_st")
                nc.sync.dma_start(
                    f_st[:], f_aps[mod][h * 128:(h + 1) * 128, :, :])
                f_sts[(mod, h)] = f_st

        # W: [1024, 512] f32 -> fp8 * WSCALE [128, 8(kblk), 512(d_out)]
        w_st = fst_pool.tile([128, 8, D], F32, tag="w_st", bufs=1)
        for kb in range(8):
            nc.sync.dma_start(w_st[:, kb, :], w_ap[kb * 128:(kb + 1) * 128, :])
        nc.vector.tensor_scalar_mul(w_f8[:], w_st[:], WSCALE)

        for mod in range(2):                   # 0 = spa, 1 = seq
            c0 = mod * HROWS
            # ---- cast f to bf16, PE-transpose, emit fT in fp8 ----
            fT = ft_pool.tile([128, 4, KB, 2 * 128], FP8, name=f"fT{mod}",
                              tag=f"fT{mod}")
            for h in range(2):                 # halves of 256 local rows
                f_st = f_sts[(mod, h)]
                f_bf = fst_pool.tile([128, 4 * D], BF16,
                                     name=f"f_bf{mod}{h}", tag="f_bf",
                                     bufs=2)
                nc.vector.tensor_copy(f_bf[:], f_st[:])
                for a in range(4):
                    for kb in range(KB):
                        pst = ps_t.tile([128, 128], BF16, name="pst",
                                        tag="pst")
                        nc.tensor.transpose(
                            pst[:],
                            f_bf[:, a * D + kb * 128: a * D + (kb + 1) * 128],
                            ident_b[:])
                        nc.scalar.activation(
                            fT[:, a, kb, h * 128:(h + 1) * 128], pst[:],
                            AF.Copy)
            # ---- projection: DoubleRow fp8, K=256 per matmul ----
            for pa in range(3):
                for m in range(KB):
                    psv = ps_proj.tile([128, 2 * 128], F32, name="psv",
                                       tag="psv")
                    for g in range(4):
                        head = LH[pa] if g < 2 else 2
                        kb = (2 * g) % 4
                        nc.tensor.matmul(
                            psv[:],
                            lhsT=w_f8[:, 2 * g:2 * g + 2,
                                      m * 128:(m + 1) * 128],
                            rhs=fT[:, head, kb:kb + 2, :],
                            start=(g == 0), stop=(g == 3), perf_mode=DR)
                    col0 = c0 + pa * 256
                    nc.vector.tensor_scalar_add(
                        vT[:, m, col0:col0 + 256], psv[:], b_col[:, m:m + 1])

            # ---- norms: ssq over d for this half's 768 columns ----
            ssq = small_pool.tile([1, HROWS], F32, name=f"ssq{mod}",
                                  tag=f"ssq{mod}")
            # spa sq muls ride the Pool queue; seq's go to DVE so they
            # are not serialized behind the spa collective_compute (which
            # blocks the Pool queue ~20us) -- pulls the seq gather's input
            # chain ~9us earlier, protecting phase C on slow-gather draws
            sq_eng = nc.gpsimd if mod == 0 else nc.vector
            for co, cw in ((0, 512), (512, 256)):
                ps_ssq = ps_s.tile([1, 512], F32, name="ps_ssq", tag="ps_s")
                for m in range(KB):
                    sq = sq_pool.tile([128, 512], BF16, name="sq", tag="sq")
                    sq_eng.tensor_mul(sq[:, :cw],
                                      vT[:, m, c0 + co:c0 + co + cw],
                                      vT[:, m, c0 + co:c0 + co + cw])
                    nc.tensor.matmul(ps_ssq[:, :cw], lhsT=ones_col_b[:],
                                     rhs=sq[:, :cw],
                                     start=(m == 0), stop=(m == KB - 1))
                nc.vector.tensor_copy(ssq[:, co:co + cw], ps_ssq[:, :cw])

            # r = ZSCALE / sqrt(ssq) = exp(-0.5*ln(ssq) + ln ZSCALE)
            # (both Ln and Exp live in the natural_log_exp ACT table set,
            #  same as the sim-phase Exp and the final Ln: one table load)
            t_ln = small_pool.tile([1, HROWS], F32, name=f"tln{mod}",
                                   tag=f"tln{mod}")
            nc.scalar.activation(t_ln[:], ssq[:], AF.Ln)
            nc.scalar.activation(r_row[:, c0:c0 + HROWS], t_ln[:], AF.Exp,
                                 bias=ln_zs[:], scale=-0.5)

            # zT_loc half = fp8(vT * r)
            rb_sb = sq_pool.tile([128, HROWS], BF16, name=f"rb{mod}",
                                 tag="rb_sb")
            for co, cw in ((0, 512), (512, 256)):
                rb = ps_s.tile([128, 512], F32, name="rbp", tag="rbp")
                nc.tensor.matmul(rb[:, :cw], lhsT=ones_row_b[:],
                                 rhs=r_row[:, c0 + co:c0 + co + cw],
                                 start=True, stop=True)
                nc.scalar.activation(rb_sb[:, co:co + cw], rb[:, :cw],
                                     AF.Copy)
            for m in range(KB):
                nc.vector.tensor_mul(
                    zT_loc[:, m, c0:c0 + HROWS],
                    vT[:, m, c0:c0 + HROWS], rb_sb[:])

        # ---- pos_i = r_i * r_{i+768} * sum_d vT[d, i] * vT[d, i+768] ----
        # (before the gather issues: with the seq sq muls on DVE, the Pool
        # queue finishes the pp muls by ~67us, so the spa collective still
        # triggers before the warm gather completes; placing pos AFTER the
        # gathers would park its PE matmuls behind the seq collective's
        # Pool-queue retirement at ~100us, stalling phase A)
        pos_raw = small_pool.tile([1, HROWS], F32, tag="pos_raw")
        for co, cw in ((0, 512), (512, 256)):
            ps_pp = ps_s.tile([1, 512], F32, name="ps_pp", tag="ps_s")
            for m in range(KB):
                pp = sq_pool.tile([128, 512], BF16, name="pp", tag="sq")
                nc.gpsimd.tensor_mul(pp[:, :cw], vT[:, m, co:co + cw],
                                     vT[:, m, HROWS + co:HROWS + co + cw])
                nc.tensor.matmul(ps_pp[:, :cw], lhsT=ones_col_b[:],
                                 rhs=pp[:, :cw],
                                 start=(m == 0), stop=(m == KB - 1))
            nc.vector.tensor_copy(pos_raw[:, co:co + cw], ps_pp[:, :cw])
        rrp = small_pool.tile([1, HROWS], F32, tag="rrp")
        nc.vector.tensor_mul(rrp[:], r_row[:, 0:HROWS], r_row[:, HROWS:LROWS])
        pos_row = small_pool.tile([1, HROWS], F32, tag="pos_row")
        nc.vector.tensor_mul(pos_row[:], pos_raw[:], rrp[:])
        pos_sum = small_pool.tile([1, 1], F32, tag="pos_sum")
        nc.vector.tensor_reduce(pos_sum[:], pos_row[:],
                                axis=mybir.AxisListType.X, op=ALU.add)

        # ---- AllGather both halves ----
        # Issued after ALL Pool-queue compute: collective_compute lives on
        # the Pool queue and blocks it until the CC core acks (~20us), so
        # any Pool op issued between the two gathers would stall and push
        # the second gather's input chain out by that much. Both ag_in
        # DMAs are issued before the unpack DMAs for the same reason (the
        # in-order sync queue).
        ag_outs = []
        for mod in range(2):
            c0 = mod * HROWS
            zT_all[mod] = big_pool.tile([128, KB, HALL], FP8,
                                        name=f"zT_all{mod}", tag=f"zTa{mod}")
            ag_in = dram_pool.tile([4 * 128, HROWS], FP8,
                                   name=f"ag_in{mod}", tag=f"ag_in{mod}")
            ag_out = dram_pool.tile([N_CORES * 4 * 128, HROWS], FP8,
                                    addr_space="Shared",
                                    name=f"ag_out{mod}",
                                    tag=f"ag_out{mod}")
            ag_outs.append(ag_out)
            nc.sync.dma_start(
                ag_in.rearrange("(m p) c -> p m c", p=128),
                zT_loc[:, :, c0:c0 + HROWS])
            nc.gpsimd.collective_compute(
                "AllGather", ALU.bypass,
                replica_groups=[list(range(N_CORES))],
                ins=[ag_in.opt()], outs=[ag_out.opt()])
        for mod in range(2):
            for rr in range(N_CORES):
                nc.sync.dma_start(
                    zT_all[mod][:, :, rr * HROWS:(rr + 1) * HROWS],
                    ag_outs[mod][rr * 512:(rr + 1) * 512, :].rearrange(
                        "(m p) c -> p m c", p=128))


    # ---------- sim tiles + fused exp/rowsum (DoubleRow fp8) ----------
    # Blocks: A = spa x spa, B = seq x seq, C = spa x seq; C' (seq rows x
    # spa cols) is never computed -- its row sums are COLUMN sums of C,
    # accumulated per-lane on DVE in bf16, partition-reduced once at the
    # end of phase C, then ReduceScattered (rank r's shard = our seq rows).
    colden = small_pool.tile([128, HIB], BF16, tag="colden")

    def sim_chunk(ps_sim, mod, ib, cc):
        ps = ps_sim.tile([128, SIMW], F32, name="ps_sim", tag="ps_sim")
        for jt in range(SIMW // MMW):
            j0 = cc * SIMW + jt * MMW
            for g in range(2):
                nc.tensor.matmul(
                    ps[:, jt * MMW:(jt + 1) * MMW],
                    lhsT=zT_loc[:, 2 * g:2 * g + 2, ib * 128:(ib + 1) * 128],
                    rhs=zT_all[mod][:, 2 * g:2 * g + 2, j0:j0 + MMW],
                    start=(g == 0), stop=(g == 1), perf_mode=DR)
        return ps

    with tc.tile_pool(name="ps_sim", bufs=2, space="PSUM") as ps_sim, \
         tc.tile_pool(name="esb", bufs=3) as esb_pool:
        # phase A: spa rows x spa cols (row sums only); cc-major so the
        # cc=0 chunks (columns entirely in spa gather half 1) run while
        # gather half 2 is still in flight
        for cc in range(CC):
            for ib in range(HIB):
                ps = sim_chunk(ps_sim, 0, ib, cc)
                scol = ib * 2 * CC + cc
                nc.scalar.activation(ps[:], ps[:], AF.Exp, scale=ESCALE,
                                     accum_out=stats[:, scol:scol + 1])
        # phase C: spa rows x seq cols (row sums + per-lane column acc)
        for ib in range(HIB):
            for cc in range(CC):
                ps = sim_chunk(ps_sim, 1, ib, cc)
                scol = ib * 2 * CC + CC + cc
                e_sb = esb_pool.tile([128, SIMW], BF16, name="e_sb",
                                     tag="e_sb")
                nc.scalar.activation(e_sb[:], ps[:], AF.Exp, scale=ESCALE,
                                     accum_out=stats[:, scol:scol + 1])
                nc.vector.tensor_add(acc_bf[:, cc, :], acc_bf[:, cc, :],
                                     e_sb[:])

        # spa-row log-denominators: complete after phase C, so this chain
        # (DVE reduce + ACT Ln + DVE reduce) hides under phase B
        denom = small_pool.tile([128, IB], F32, tag="denom")
        logd = small_pool.tile([128, IB], F32, tag="logd")
        logsum = small_pool.tile([128, 3], F32, tag="logsum")
        nc.vector.tensor_reduce(
            denom[:, 0:HIB],
            stats[:, 0:HIB * 2 * CC].rearrange("p (i x) -> p i x", x=2 * CC),
            axis=mybir.AxisListType.X, op=ALU.add)
        nc.scalar.activation(logd[:, 0:HIB], denom[:, 0:HIB], AF.Ln,
                             bias=neg_e2[:])
        nc.vector.tensor_reduce(logsum[:, 0:1], logd[:, 0:HIB],
                                axis=mybir.AxisListType.X, op=ALU.add)

        # partition-reduce the column accumulator into a ps_sim-pool tile
        # (no extra PSUM bank, no pool-scope transition), ReduceScatter it
        rs_in = dram_pool.tile([HALL], BF16, tag="rs_in")
        rs_out = dram_pool.tile([HROWS], BF16, tag="rs_out")
        colacc = small_pool.tile([1, HALL], BF16, tag="colacc")
        for cc in range(CC):
            pcps = ps_sim.tile([128, SIMW], F32, name="pcps", tag="ps_sim")
            for jt in range(JT):
                nc.tensor.matmul(pcps[0:1, jt * 512:jt * 512 + 512],
                                 lhsT=ones_col_b[:],
                                 rhs=acc_bf[:, cc, jt * 512:(jt + 1) * 512],
                                 start=True, stop=True)
            nc.vector.tensor_copy(colacc[:, cc * SIMW:(cc + 1) * SIMW],
                                  pcps[0:1, :])
        nc.sync.dma_start(rs_in[:], colacc[:])
        nc.gpsimd.collective_compute(
            "ReduceScatter", ALU.add,
            replica_groups=[list(range(N_CORES))],
            ins=[rs_in.opt()], outs=[rs_out.opt()])
        for j in range(HIB):
            nc.sync.dma_start(colden[:, j:j + 1],
                              rs_out[j * 128:(j + 1) * 128])

        # phase B: seq rows x seq cols (row sums only). The log-denominator
        # chain for ib 6..10 is issued after ib 10 so it hides under ib 11's
        # chunks; only ib 11's chain remains on the tail.
        for ib in range(HIB, IB):
            for cc in range(CC):
                ps = sim_chunk(ps_sim, 1, ib, cc)
                scol = ib * 2 * CC + CC + cc
                nc.scalar.activation(ps[:], ps[:], AF.Exp, scale=ESCALE,
                                     accum_out=stats[:, scol:scol + 1])
            if ib == IB - 2:
                # pre-reduce ib 6..10 row-sum stats on DVE (no ACT, no
                # colden dependency -- those stay on the short tail)
                s0, s1 = HIB * 2 * CC, (IB - 1) * 2 * CC
                nc.vector.tensor_reduce(
                    denom[:, HIB:IB - 1],
                    stats[:, s0:s1].rearrange("p (i x) -> p i x", x=2 * CC),
                    axis=mybir.AxisListType.X, op=ALU.add)

    # ---------- final reduction (seq rows + combine) ----------
    with tc.tile_pool(name="ps_fin", bufs=1, space="PSUM") as ps_fin:
        s1 = (IB - 1) * 2 * CC
        nc.vector.tensor_reduce(
            denom[:, IB - 1:IB],
            stats[:, s1:].rearrange("p (i x) -> p i x", x=2 * CC),
            axis=mybir.AxisListType.X, op=ALU.add)
        # seq rows: add the ReduceScattered spa-column contributions
        nc.vector.tensor_add(denom[:, HIB:IB], denom[:, HIB:IB], colden[:])
        nc.scalar.activation(logd[:, HIB:IB], denom[:, HIB:IB], AF.Ln,
                             bias=neg_e2[:])
        nc.vector.tensor_reduce(logsum[:, 1:2], logd[:, HIB:IB],
                                axis=mybir.AxisListType.X, op=ALU.add)
        fin = ps_fin.tile([1, 2], F32, tag="fin")
        nc.tensor.matmul(fin[:], lhsT=ones_col[:], rhs=logsum[:, 0:2],
                         start=True, stop=True)
        res = small_pool.tile([1, 1], F32, tag="res")
        # res = (pos_sum * POS_COEF + sum(log denom)) / R
        nc.vector.scalar_tensor_tensor(res[:], pos_sum[:], POS_COEF,
                                       fin[:, 0:1], op0=ALU.mult, op1=ALU.add)
        nc.vector.tensor_add(res[:], res[:], fin[:, 1:2])
        nc.vector.tensor_scalar_mul(res[:], res[:], INV_COUNT)
        nc.sync.dma_start(out_ap[:], res[:])


_NC_CACHE = None


def build_nc():
    global _NC_CACHE
    if _NC_CACHE is not None:
        return _NC_CACHE
    nc = bacc.Bacc("TRN2", target_bir_lowering=False, debug=False,
                   num_devices=N_CORES)
    f_spa = nc.dram_tensor("f_spa", [BL, 4, D], F32, kind="ExternalInput").ap()
    f_seq = nc.dram_tensor("f_seq", [BL, 4, D], F32, kind="ExternalInput").ap()
    w_ap = nc.dram_tensor("W", [2 * D, D], F32, kind="ExternalInput").ap()
    b_ap = nc.dram_tensor("b", [D], F32, kind="ExternalInput").ap()
    out_ap = nc.dram_tensor("out", [1, 1], F32, kind="ExternalOutput").ap()
    with tile.TileContext(nc) as tc, ExitStack() as ctx:
        _body(ctx, nc, tc, (f_spa, f_seq), w_ap, b_ap, out_ap)
    nc.compile()
    _NC_CACHE = nc
    return nc


def run(inputs, **kw):
    nc = build_nc()
    f_seq = np.ascontiguousarray(np.asarray(inputs["f_seq"], dtype=np.float32))
    f_spa = np.ascontiguousarray(np.asarray(inputs["f_spa"], dtype=np.float32))
    W = np.ascontiguousarray(np.asarray(inputs["W"], dtype=np.float32))
    b = np.ascontiguousarray(np.asarray(inputs["b"], dtype=np.float32))
    in_maps = []
    for c in range(N_CORES):
        sl = slice(c * BL, (c + 1) * BL)
        in_maps.append({"f_seq": np.ascontiguousarray(f_seq[sl]),
                        "f_spa": np.ascontiguousarray(f_spa[sl]),
                        "W": W, "b": b})
    try:
        res = bass_utils.run_bass_kernel_spmd(
            nc, in_maps, core_ids=list(range(N_CORES)), **kw)
    except Exception:
        # the axon terminal occasionally reports a transient
        # "device unrecoverable" on first attach; one retry clears it
        import time
        time.sleep(15)
        res = bass_utils.run_bass_kernel_spmd(
            nc, in_maps, core_ids=list(range(N_CORES)), **kw)
    total = np.float64(0.0)
    for c in range(N_CORES):
        total += np.float64(res.results[c]["out"][0, 0])
    return np.float32(total), res


def kernel(**inputs) -> np.ndarray:
    loss, _ = run(inputs)
    return np.asarray(loss, dtype=np.float32)


if __name__ == "__main__":
    rng = np.random.default_rng(0)
    inputs = {
        "f_seq": rng.standard_normal((B, 4, D), dtype=np.float32),
        "f_spa": rng.standard_normal((B, 4, D), dtype=np.float32),
        "W": (rng.standard_normal((2 * D, D), dtype=np.float32) * 0.02),
        "b": np.zeros((D,), dtype=np.float32),
    }
    print(kernel(**inputs))
